# revision 4
# baseline (speedup 1.0000x reference)
"""BoundaryLoss kernel v3: EDT min-plus passes as PE band-matmuls in the exp
domain (see v2 notes below), plus a dispatch path built for a high-latency
PJRT tunnel.

Math (unchanged from v2):
  S2[x,y] = sum_{|j|,|k|<=4} 2^(-5(j^2+k^2)) * bg[y+k, x+j]
          = 2^(-5*d2) * (1+R),  R < 0.4
  => floor(log2(S2)) recovers -5*d2 exactly from the f32 exponent bits.
Both band convolutions are matmuls with 128x128 banded matrices (weights are
exact powers of two in bf16); the x-direction pass runs on the transposed
intermediate.

v3 changes, all aimed at wall-clock per call through the axon tunnel
(~71 ms RTT + ~60 MB/s marginal transfer bandwidth):
  - the jitted shard_map executable is built once and cached; the stock
    run_bass_kernel_spmd re-traces and re-jits a fresh closure every call
    (~165 ms/call).
  - pred ships as fp8 e4m3 (64 KB/core) instead of f32 (256 KB/core); adds
    ~7e-4 relative error, loss tail already runs in bf16.
  - the 0/1 mask ships bit-packed (8 KB/core) and is unpacked on-device
    with 8 shift/and ops.
  - the banded weight matrix is generated on-device (iota -> square -> Exp)
    instead of shipping 192 KB/core of constants.
"""

import math

import numpy as np

import concourse.bass as bass
import concourse.tile as tile
from concourse import bacc, mybir

H = W = 256
P = 128
K = 4
BETA_LOG2 = 5          # base 2^-5
N_CORES = 8

F32 = mybir.dt.float32
BF16 = mybir.dt.bfloat16
I32 = mybir.dt.int32
U8 = mybir.dt.uint8
FP8 = mybir.dt.float8e4
ALU = mybir.AluOpType
ACTF = mybir.ActivationFunctionType

LN2 = math.log(2.0)


def _band_pass(nc, out_psum, band, rhs, c0):
    """out_psum[:, t, :] = band-conv along the partition dim of rhs chunks
    [c0, c0+2). out_psum: [P, 2, W] psum f32; rhs: [P, 4, W] bf16 sbuf.
    band slots: 0 = edgeUp (in tile1 -> out tile0), 1 = main,
    2 = edgeDn (in tile0 -> out tile1)."""
    for t in (0, 1):
        o = out_psum[:, t, :]
        nc.tensor.matmul(o, band[:, 1, :], rhs[:, c0 + t, :],
                         start=True, stop=False)
        edge = band[:, 0, :] if t == 0 else band[:, 2, :]
        other = rhs[:, c0 + (1 - t), :]
        nc.tensor.matmul(o, edge, other, start=False, stop=True)


def _build_body(nc, tc, pool, psum_pool, pred_d, ch0b_d, out_d):
    # packed mask bits: row y -> bytes [32*y, 32*(y+1)), little bit order
    bits = pool.tile([P, 2, 32], U8)
    nc.sync.dma_start(bits[:], ch0b_d.ap().rearrange("(t p) b -> p t b", p=P))
    predf8 = pool.tile([P, 2, W], FP8)
    nc.scalar.dma_start(predf8[:], pred_d.ap().rearrange("(t p) x -> p t x", p=P))

    # band weights on-device: d[p,c,j] = 128*(c-1) + j - p, w = 2^(-5*d^2).
    # |d|>4 underflows to ~2^-125 (bf16 normal min is 2^-126) or 0 -- either
    # is far below the smallest legit S2 term, so the tail never perturbs
    # the recovered exponent.
    di = pool.tile([P, 3, P], F32)
    nc.gpsimd.iota(di[:], [[P, 3], [1, P]], base=-P, channel_multiplier=-1,
                   allow_small_or_imprecise_dtypes=True)
    sq = pool.tile([P, 3, P], F32)
    nc.gpsimd.tensor_tensor(sq[:], di[:], di[:], ALU.mult)
    band = pool.tile([P, 3, P], BF16)
    nc.scalar.activation(band[:], sq[:], ACTF.Exp, scale=-BETA_LOG2 * LN2)

    # unpack bits -> 0/1 u8; ch0u[p, t, b, j] = bit j of bits[p, t, b]
    ch0u = pool.tile([P, 2, 32, 8], U8)
    for j in range(8):
        nc.vector.tensor_scalar(ch0u[:, :, :, j], bits[:], j, 1,
                                ALU.logical_shift_right, ALU.bitwise_and)
    c0 = ch0u[:].rearrange("p t b j -> p t (b j)")

    # masks: chunks 0,1 = A (bg = neg = ch0), 2,3 = B (bg = pos = 1-ch0)
    m = pool.tile([P, 4, W], BF16)
    nc.vector.tensor_copy(m[:, 0:2, :], c0)
    nc.vector.tensor_scalar(m[:, 2:4, :], c0, -1.0, -1.0,
                            ALU.mult, ALU.subtract)   # 1 - ch0

    predb = pool.tile([P, 2, W], BF16)
    nc.gpsimd.tensor_copy(predb[:], predf8[:])

    # pass1: y-direction band conv (layout A) -> T1 (psum) -> bf16 sbuf
    t1p = psum_pool.tile([P, 2, W], F32, tag="t1a")
    t1pb = psum_pool.tile([P, 2, W], F32, tag="t1b")
    t1 = pool.tile([P, 4, W], BF16)
    _band_pass(nc, t1pb, band, m, 2)     # mask B first
    nc.vector.tensor_copy(t1[:, 2:4, :], t1pb[:])
    _band_pass(nc, t1p, band, m, 0)      # mask A
    nc.vector.tensor_copy(t1[:, 0:2, :], t1p[:])

    # transpose t1 chunks (mask, ytile) -> (mask, xtile); also pred
    t1T = pool.tile([P, 4, W], BF16)
    slot = 0
    for mm in (1, 0):
        for yt in (0, 1):
            for xb in (0, 1):
                eng = nc.sync if slot % 2 == 0 else nc.scalar
                eng.dma_start_transpose(
                    t1T[:, 2 * mm + xb, P * yt:P * (yt + 1)],
                    t1[:, 2 * mm + yt, P * xb:P * (xb + 1)])
                slot += 1
    predT = pool.tile([P, 2, W], BF16)
    for yt in (0, 1):
        for xb in (0, 1):
            eng = nc.sync if slot % 2 == 0 else nc.scalar
            eng.dma_start_transpose(
                predT[:, xb, P * yt:P * (yt + 1)],
                predb[:, yt, P * xb:P * (xb + 1)])
            slot += 1

    # pass2: x-direction band conv (layout B) -> S2 (psum f32)
    s2b = psum_pool.tile([P, 2, W], F32, tag="s2b")
    s2a = psum_pool.tile([P, 2, W], F32, tag="s2a")
    _band_pass(nc, s2b, band, t1T, 2)
    _band_pass(nc, s2a, band, t1T, 0)

    # recovery: exponent(S2)-127 = -5*d2 + floor(log2 mass), mass in [1,13]
    # (multiple equidistant bg pixels add mass).  t = 131-eb = 5*d2+(4-di),
    # di in {0..3}; 2^(t/5) = 2^(d2+0.2..0.8), whose exponent is exactly d2.
    LN2_5 = LN2 / BETA_LOG2
    bcon = pool.tile([P, 2], F32)
    nc.gpsimd.memset(bcon[:, 0:1], 131.0 * LN2_5)
    nc.gpsimd.memset(bcon[:, 1:2], -127.0)
    e5a = pool.tile([P, 2, W], F32)
    e5b = pool.tile([P, 2, W], F32)
    # arith op casts int32->f32: v*2^-23 = eb + mant_frac, frac in [0,0.56)
    nc.vector.tensor_scalar(e5b[:], s2b[:].bitcast(I32), 2.0 ** -23, None,
                            ALU.mult)
    nc.vector.tensor_scalar(e5a[:], s2a[:].bitcast(I32), 2.0 ** -23, None,
                            ALU.mult)
    ga = pool.tile([P, 2, W], F32)
    gb = pool.tile([P, 2, W], F32)
    nc.scalar.activation(gb[:], e5b[:], ACTF.Exp, scale=-LN2_5,
                         bias=bcon[:, 0:1])  # 2^((131-eb)/5)
    nc.scalar.activation(ga[:], e5a[:], ACTF.Exp, scale=-LN2_5,
                         bias=bcon[:, 0:1])
    d2sa = pool.tile([P, 2, W], I32)
    d2sb = pool.tile([P, 2, W], I32)
    nc.vector.tensor_scalar(d2sb[:], gb[:].bitcast(I32), 23, None,
                            ALU.arith_shift_right)   # i32 -> i32, no cast
    nc.vector.tensor_scalar(d2sa[:], ga[:].bitcast(I32), 23, None,
                            ALU.arith_shift_right)
    d2ia = pool.tile([P, 2, W], BF16)
    d2ib = pool.tile([P, 2, W], BF16)
    nc.vector.tensor_copy(d2ib[:], d2sb[:])
    nc.vector.tensor_copy(d2ia[:], d2sa[:])
    aA = pool.tile([P, 2, W], BF16)
    aB = pool.tile([P, 2, W], BF16)
    nc.scalar.activation(aB[:], d2ib[:], ACTF.Sqrt, bias=bcon[:, 1:2])
    nc.scalar.activation(aA[:], d2ia[:], ACTF.Sqrt, bias=bcon[:, 1:2])

    sdt = pool.tile([P, 2, W], BF16)
    nc.vector.tensor_tensor(sdt[:], aA[:], aB[:], ALU.subtract)
    sabs = pool.tile([P, 2, W], BF16)
    nc.gpsimd.tensor_tensor(sabs[:], aA[:], aB[:], ALU.add)
    wgt = pool.tile([P, 2, W], BF16)
    nc.scalar.activation(wgt[:], sabs[:], ACTF.Exp, scale=-0.2)
    t = pool.tile([P, 2, W], BF16)
    nc.vector.tensor_tensor(t[:], predT[:], sdt[:], ALU.subtract)
    tabs = pool.tile([P, 2, W], BF16)
    nc.vector.scalar_tensor_tensor(tabs[:], t[:], -1.0, t[:],
                                   ALU.mult, ALU.max)
    scr = pool.tile([P, 2, W], BF16)
    acc = pool.tile([P, 1], F32)
    nc.vector.scalar_tensor_tensor(scr[:], tabs[:], 0.0, wgt[:],
                                   ALU.add, ALU.mult, accum_out=acc[:])

    ones = pool.tile([P, 1], F32)
    nc.gpsimd.memset(ones[:], 1.0)
    red = psum_pool.tile([1, 1], F32, tag="red")
    nc.tensor.matmul(red[:], acc[:], ones[:], start=True, stop=True)
    sb = pool.tile([1, 1], F32)
    nc.vector.tensor_copy(sb[:], red[:])
    nc.sync.dma_start(out_d.ap(), sb[:])


def build_nc():
    nc = bacc.Bacc("TRN2", debug=False, enable_asserts=False,
                   num_devices=N_CORES)
    pred_d = nc.dram_tensor("pred", [H, W], FP8, kind="ExternalInput")
    ch0b_d = nc.dram_tensor("ch0b", [H, 32], U8, kind="ExternalInput")
    out_d = nc.dram_tensor("out", [1, 1], F32, kind="ExternalOutput")
    with tile.TileContext(nc) as tc:
        with (
            tc.tile_pool(name="main", bufs=1) as pool,
            tc.tile_pool(name="ps", bufs=1, space="PSUM") as psum_pool,
        ):
            _build_body(nc, tc, pool, psum_pool, pred_d, ch0b_d, out_d)
    nc.compile()
    return nc


_NC = None
_RUN = None


def get_nc():
    global _NC
    if _NC is None:
        _NC = build_nc()
    return _NC


def _build_runner():
    """One-time: jit the shard_map'd bass executable over 8 cores. The stock
    run_bass_kernel_spmd builds a fresh closure (and thus a fresh jit cache
    entry) per call; caching this saves ~165 ms/call."""
    import jax
    from jax.sharding import Mesh, PartitionSpec
    from jax.experimental.shard_map import shard_map
    from concourse import bass2jax

    nc = get_nc()
    bass2jax.install_neuronx_cc_hook()

    partition_name = (nc.partition_id_tensor.name
                      if nc.partition_id_tensor else None)
    in_names, out_names, out_avals = [], [], []
    for alloc in nc.m.functions[0].allocations:
        if not isinstance(alloc, mybir.MemoryLocationSet):
            continue
        name = alloc.memorylocations[0].name
        if alloc.kind == "ExternalInput":
            if name != partition_name:
                in_names.append(name)
        elif alloc.kind == "ExternalOutput":
            out_names.append(name)
            out_avals.append(jax.core.ShapedArray(
                tuple(alloc.tensor_shape), mybir.dt.np(alloc.dtype)))

    n_params = len(in_names)
    all_names = list(in_names) + list(out_names)
    if partition_name is not None:
        all_names.append(partition_name)
    donate = tuple(range(n_params, n_params + len(out_names)))

    def _body(*args):
        operands = list(args)
        if partition_name is not None:
            operands.append(bass2jax.partition_id_tensor())
        outs = bass2jax._bass_exec_p.bind(
            *operands,
            out_avals=tuple(out_avals),
            in_names=tuple(all_names),
            out_names=tuple(out_names),
            lowering_input_output_aliases=(),
            sim_require_finite=True,
            sim_require_nnan=True,
            nc=nc,
        )
        return tuple(outs)

    devices = jax.devices()[:N_CORES]
    mesh = Mesh(np.asarray(devices), ("core",))
    nspec = n_params + len(out_names)
    jitted = jax.jit(
        shard_map(_body, mesh=mesh,
                  in_specs=(PartitionSpec("core"),) * nspec,
                  out_specs=(PartitionSpec("core"),) * len(out_names),
                  check_rep=False),
        donate_argnums=donate, keep_unused=True,
    )
    zero_shapes = [((N_CORES * a.shape[0],) + tuple(a.shape[1:]), a.dtype)
                   for a in out_avals]

    def run(in_by_name):
        args = [in_by_name[nm] for nm in in_names]
        args += [np.zeros(s, d) for s, d in zero_shapes]
        outs = jitted(*args)
        return [np.asarray(o) for o in outs]

    return run


def get_runner():
    global _RUN
    if _RUN is None:
        _RUN = _build_runner()
    return _RUN


_FP8_LUT = None


def _fp8_lut():
    """bf16-pattern -> fp8 byte table: f32 -> (trunc) bf16 -> (rne) fp8.
    One gather beats ml_dtypes' elementwise astype ~2x on 512K elements."""
    global _FP8_LUT
    if _FP8_LUT is None:
        v = (np.arange(65536, dtype=np.uint32) << 16).view(np.float32)
        with np.errstate(invalid="ignore"):
            _FP8_LUT = v.astype(mybir.dt.np(FP8)).view(np.uint8)
    return _FP8_LUT


def kernel(pred_sdt: np.ndarray, target_seg: np.ndarray) -> np.ndarray:
    run = get_runner()
    pred = np.ascontiguousarray(np.asarray(pred_sdt), dtype=np.float32)
    idx = (pred.view(np.uint32) >> 16).astype(np.uint16)
    pred8 = _fp8_lut()[idx].view(mybir.dt.np(FP8)).reshape(N_CORES * H, W)
    # target_seg channel 0 holds exact 0.0f/1.0f; byte 2 of the f32 LE
    # pattern (0x80 for 1.0) is a nonzero test packbits accepts directly.
    segb = np.asarray(target_seg).view(np.uint8)
    ch0_nz = segb.reshape(N_CORES, 3, H, W, 4)[:, 0, :, :, 2]
    bits = np.packbits(ch0_nz, bitorder="little").reshape(N_CORES * H, 32)
    (out,) = run({"pred": pred8, "ch0b": bits})
    return np.float32(float(out.sum()) / (N_CORES * H * W))


# revision 11
# speedup vs baseline: 1.6038x; 1.6038x over previous
"""BoundaryLoss kernel v5: EDT min-plus passes as PE band-matmuls in the
exp domain; single-core dispatch and 4-bit pred, tuned for a high-latency
PJRT tunnel.

Math (unchanged from v2):
  S2[x,y] = sum_{|j|,|k|<=4} 2^(-5(j^2+k^2)) * bg[y+k, x+j]
          = 2^(-5*d2) * (1+R),  R < 0.4
  => floor(log2(S2)) recovers -5*d2 exactly from the f32 exponent bits.
Both band convolutions are matmuls with 128x128 banded matrices (weights are
exact powers of two in bf16); the x-direction pass runs on the transposed
intermediate.

Dispatch rationale (measured): every call through the tunnel pays a
~50-70 ms base RTT regardless of payload; marginal transfer bandwidth is
~90-110 MB/s; each *per-device* H2D shard transfer costs ~1.5 ms extra. An
8-core shard_map loses ~12 ms to shard coordination while the on-device
compute for all 8 images is ~0.13 ms (TimelineSim). So: ONE core runs all 8
images sequentially, inputs arrive as two small host-packed arrays (4-bit
quantized pred 256 KB + bit-packed mask 64 KB, unpacked on-device), and the
jitted executable is built once and cached (the stock run_bass_kernel_spmd
re-jits a fresh closure every call, ~165 ms/call). 4-bit pred adds ~4e-3
relative error against a 2e-2 budget (pred enters only through
|pred - sdt| * w; quantization noise largely cancels in the mean).
"""

import math

import numpy as np

import concourse.bass as bass
import concourse.tile as tile
from concourse import bacc, mybir

H = W = 256
P = 128
K = 4
BETA_LOG2 = 5          # base 2^-5
B = 8                  # images, all on core 0

F32 = mybir.dt.float32
BF16 = mybir.dt.bfloat16
I32 = mybir.dt.int32
U8 = mybir.dt.uint8
FP8 = mybir.dt.float8e4
ALU = mybir.AluOpType
ACTF = mybir.ActivationFunctionType

LN2 = math.log(2.0)
QSPAN = 12.0                      # pred quant range [-QSPAN, QSPAN]
QSTEP = 2 * QSPAN / 15            # 16 uniform levels at (k-7.5)*QSTEP


def _band_pass(nc, out_psum, band, rhs, c0):
    """out_psum[:, t, :] = band-conv along the partition dim of rhs chunks
    [c0, c0+2). out_psum: [P, 2, W] psum f32; rhs: [P, 4, W] bf16 sbuf.
    band slots: 0 = edgeUp (in tile1 -> out tile0), 1 = main,
    2 = edgeDn (in tile0 -> out tile1)."""
    for t in (0, 1):
        o = out_psum[:, t, :]
        nc.tensor.matmul(o, band[:, 1, :], rhs[:, c0 + t, :],
                         start=True, stop=False)
        edge = band[:, 0, :] if t == 0 else band[:, 2, :]
        other = rhs[:, c0 + (1 - t), :]
        nc.tensor.matmul(o, edge, other, start=False, stop=True)


def _image_body(nc, b, st, band, pred_d, ch0b_d):
    """Emit one image's pipeline using the preallocated tile set `st`.
    Accumulates the image's weighted-L1 sum into st.acc_all[:, b]."""
    bits, prednib = st["bits"], st["prednib"]
    nc.sync.dma_start(
        bits[:], ch0b_d.ap()[b * H:(b + 1) * H, :]
        .rearrange("(t p) b -> p t b", p=P))
    nc.scalar.dma_start(
        prednib[:], pred_d.ap()[b * H:(b + 1) * H, :]
        .rearrange("(t p) x -> p t x", p=P))

    # unpack bits -> 0/1 u8; ch0u[p, t, byte, j] = bit j of bits[p, t, byte]
    ch0u = st["ch0u"]
    for j in range(8):
        nc.vector.tensor_scalar(ch0u[:, :, :, j], bits[:], j, 1,
                                ALU.logical_shift_right, ALU.bitwise_and)
    c0 = ch0u[:].rearrange("p t b j -> p t (b j)")

    # masks: chunks 0,1 = A (bg = neg = ch0), 2,3 = B (bg = pos = 1-ch0)
    m = st["m"]
    nc.vector.tensor_copy(m[:, 0:2, :], c0)
    nc.vector.tensor_scalar(m[:, 2:4, :], c0, -1.0, -1.0,
                            ALU.mult, ALU.subtract)   # 1 - ch0

    # unpack nibbles -> dequant: val = (nib - 7.5) * QSTEP, straight to bf16
    punp = st["punp"]
    nc.vector.tensor_scalar(punp[:, :, :, 0], prednib[:], 15, None,
                            ALU.bitwise_and)
    nc.vector.tensor_scalar(punp[:, :, :, 1], prednib[:], 4, 15,
                            ALU.logical_shift_right, ALU.bitwise_and)
    predb = st["predb"]
    nc.vector.tensor_scalar(predb[:], punp[:].rearrange("p t b j -> p t (b j)"),
                            -7.5, QSTEP, ALU.add, ALU.mult)

    # pass1: y-direction band conv (layout A) -> T1 (psum) -> bf16 sbuf
    t1p, t1pb, t1 = st["t1p"], st["t1pb"], st["t1"]
    _band_pass(nc, t1pb, band, m, 2)     # mask B first
    nc.vector.tensor_copy(t1[:, 2:4, :], t1pb[:])
    _band_pass(nc, t1p, band, m, 0)      # mask A
    nc.vector.tensor_copy(t1[:, 0:2, :], t1p[:])

    # transpose t1 chunks (mask, ytile) -> (mask, xtile); also pred
    t1T, predT = st["t1T"], st["predT"]
    slot = 0
    for mm in (1, 0):
        for yt in (0, 1):
            for xb in (0, 1):
                eng = nc.sync if slot % 2 == 0 else nc.scalar
                eng.dma_start_transpose(
                    t1T[:, 2 * mm + xb, P * yt:P * (yt + 1)],
                    t1[:, 2 * mm + yt, P * xb:P * (xb + 1)])
                slot += 1
    for yt in (0, 1):
        for xb in (0, 1):
            eng = nc.sync if slot % 2 == 0 else nc.scalar
            eng.dma_start_transpose(
                predT[:, xb, P * yt:P * (yt + 1)],
                predb[:, yt, P * xb:P * (xb + 1)])
            slot += 1

    # pass2: x-direction band conv (layout B) -> S2 (psum f32)
    s2b, s2a = st["s2b"], st["s2a"]
    _band_pass(nc, s2b, band, t1T, 2)
    _band_pass(nc, s2a, band, t1T, 0)

    # recovery: exponent(S2)-127 = -5*d2 + floor(log2 mass), mass in [1,13]
    # (multiple equidistant bg pixels add mass).  t = 131-eb = 5*d2+(4-di),
    # di in {0..3}; 2^(t/5) = 2^(d2+0.2..0.8), whose exponent is exactly d2.
    LN2_5 = LN2 / BETA_LOG2
    bcon = st["bcon"]
    e5a, e5b = st["e5a"], st["e5b"]
    # arith op casts int32->f32: v*2^-23 = eb + mant_frac, frac in [0,0.56)
    nc.vector.tensor_scalar(e5b[:], s2b[:].bitcast(I32), 2.0 ** -23, None,
                            ALU.mult)
    nc.vector.tensor_scalar(e5a[:], s2a[:].bitcast(I32), 2.0 ** -23, None,
                            ALU.mult)
    ga, gb = st["ga"], st["gb"]
    nc.scalar.activation(gb[:], e5b[:], ACTF.Exp, scale=-LN2_5,
                         bias=bcon[:, 0:1])  # 2^((131-eb)/5)
    nc.scalar.activation(ga[:], e5a[:], ACTF.Exp, scale=-LN2_5,
                         bias=bcon[:, 0:1])
    d2sa, d2sb = st["d2sa"], st["d2sb"]
    nc.vector.tensor_scalar(d2sb[:], gb[:].bitcast(I32), 23, None,
                            ALU.arith_shift_right)   # i32 -> i32, no cast
    nc.vector.tensor_scalar(d2sa[:], ga[:].bitcast(I32), 23, None,
                            ALU.arith_shift_right)
    d2ia, d2ib = st["d2ia"], st["d2ib"]
    nc.vector.tensor_copy(d2ib[:], d2sb[:])
    nc.vector.tensor_copy(d2ia[:], d2sa[:])
    aA, aB = st["aA"], st["aB"]
    nc.scalar.activation(aB[:], d2ib[:], ACTF.Sqrt, bias=bcon[:, 1:2])
    nc.scalar.activation(aA[:], d2ia[:], ACTF.Sqrt, bias=bcon[:, 1:2])

    sdt, sabs, wgt, t, tabs, scr = (st["sdt"], st["sabs"], st["wgt"],
                                    st["t"], st["tabs"], st["scr"])
    nc.vector.tensor_tensor(sdt[:], aA[:], aB[:], ALU.subtract)
    nc.gpsimd.tensor_tensor(sabs[:], aA[:], aB[:], ALU.add)
    nc.scalar.activation(wgt[:], sabs[:], ACTF.Exp, scale=-0.2)
    nc.vector.tensor_tensor(t[:], predT[:], sdt[:], ALU.subtract)
    nc.vector.scalar_tensor_tensor(tabs[:], t[:], -1.0, t[:],
                                   ALU.mult, ALU.max)
    nc.vector.scalar_tensor_tensor(scr[:], tabs[:], 0.0, wgt[:],
                                   ALU.add, ALU.mult,
                                   accum_out=st["acc_all"][:, b:b + 1])


def _build_body(nc, tc, pool, psum_pool, pred_d, ch0b_d, out_d):
    # band weights on-device: d[p,c,j] = 128*(c-1) + j - p, w = 2^(-5*d^2).
    # |d|>4 underflows to ~2^-125 (bf16 normal min is 2^-126) or 0 -- far
    # below the smallest legit S2 term, so the tail never perturbs the
    # recovered exponent.
    di = pool.tile([P, 3, P], F32)
    nc.gpsimd.iota(di[:], [[P, 3], [1, P]], base=-P, channel_multiplier=-1,
                   allow_small_or_imprecise_dtypes=True)
    sq = pool.tile([P, 3, P], F32)
    nc.gpsimd.tensor_tensor(sq[:], di[:], di[:], ALU.mult)
    band = pool.tile([P, 3, P], BF16)
    nc.scalar.activation(band[:], sq[:], ACTF.Exp, scale=-BETA_LOG2 * LN2)

    bcon = pool.tile([P, 2], F32)
    nc.gpsimd.memset(bcon[:, 0:1], 131.0 * (LN2 / BETA_LOG2))
    nc.gpsimd.memset(bcon[:, 1:2], -127.0)

    # one reusable tile set; the Tile framework serializes cross-image
    # reuse hazards. Two psum tile pairs per pass stay within PSUM budget.
    st = {
        "bits": pool.tile([P, 2, 32], U8),
        "predf8": pool.tile([P, 2, W], FP8),
        "ch0u": pool.tile([P, 2, 32, 8], U8),
        "m": pool.tile([P, 4, W], BF16),
        "predb": pool.tile([P, 2, W], BF16),
        "t1": pool.tile([P, 4, W], BF16),
        "t1T": pool.tile([P, 4, W], BF16),
        "predT": pool.tile([P, 2, W], BF16),
        "e5a": pool.tile([P, 2, W], F32),
        "e5b": pool.tile([P, 2, W], F32),
        "ga": pool.tile([P, 2, W], F32),
        "gb": pool.tile([P, 2, W], F32),
        "d2sa": pool.tile([P, 2, W], I32),
        "d2sb": pool.tile([P, 2, W], I32),
        "d2ia": pool.tile([P, 2, W], BF16),
        "d2ib": pool.tile([P, 2, W], BF16),
        "aA": pool.tile([P, 2, W], BF16),
        "aB": pool.tile([P, 2, W], BF16),
        "sdt": pool.tile([P, 2, W], BF16),
        "sabs": pool.tile([P, 2, W], BF16),
        "wgt": pool.tile([P, 2, W], BF16),
        "t": pool.tile([P, 2, W], BF16),
        "tabs": pool.tile([P, 2, W], BF16),
        "scr": pool.tile([P, 2, W], BF16),
        "acc_all": pool.tile([P, B], F32),
        "bcon": bcon,
        "t1p": psum_pool.tile([P, 2, W], F32, tag="t1a", name="t1p"),
        "t1pb": psum_pool.tile([P, 2, W], F32, tag="t1b", name="t1pb"),
        "s2b": psum_pool.tile([P, 2, W], F32, tag="s2b", name="s2b"),
        "s2a": psum_pool.tile([P, 2, W], F32, tag="s2a", name="s2a"),
    }

    for b in range(B):
        _image_body(nc, b, st, band, pred_d, ch0b_d)

    ones = pool.tile([P, 1], F32)
    nc.gpsimd.memset(ones[:], 1.0)
    red = psum_pool.tile([B, 1], F32, tag="red")
    nc.tensor.matmul(red[:], st["acc_all"][:], ones[:], start=True, stop=True)
    sb = pool.tile([B, 1], F32)
    nc.vector.tensor_copy(sb[:], red[:])
    nc.sync.dma_start(out_d.ap(), sb[:])


def build_nc():
    nc = bacc.Bacc("TRN2", debug=False, enable_asserts=False,
                   num_devices=1)
    pred_d = nc.dram_tensor("pred", [B * H, W // 2], U8, kind="ExternalInput")
    ch0b_d = nc.dram_tensor("ch0b", [B * H, 32], U8, kind="ExternalInput")
    out_d = nc.dram_tensor("out", [B, 1], F32, kind="ExternalOutput")
    with tile.TileContext(nc) as tc:
        with (
            tc.tile_pool(name="main", bufs=1) as pool,
            tc.tile_pool(name="ps", bufs=1, space="PSUM") as psum_pool,
        ):
            _build_body(nc, tc, pool, psum_pool, pred_d, ch0b_d, out_d)
    nc.compile()
    return nc


_NC = None
_RUN = None


def get_nc():
    global _NC
    if _NC is None:
        _NC = build_nc()
    return _NC


def _build_runner():
    """One-time: jit the single-device bass executable and cache it."""
    import jax
    from concourse import bass2jax

    nc = get_nc()
    bass2jax.install_neuronx_cc_hook()

    partition_name = (nc.partition_id_tensor.name
                      if nc.partition_id_tensor else None)
    in_names, out_names, out_avals = [], [], []
    for alloc in nc.m.functions[0].allocations:
        if not isinstance(alloc, mybir.MemoryLocationSet):
            continue
        name = alloc.memorylocations[0].name
        if alloc.kind == "ExternalInput":
            if name != partition_name:
                in_names.append(name)
        elif alloc.kind == "ExternalOutput":
            out_names.append(name)
            out_avals.append(jax.core.ShapedArray(
                tuple(alloc.tensor_shape), mybir.dt.np(alloc.dtype)))

    n_params = len(in_names)
    all_names = list(in_names) + list(out_names)
    if partition_name is not None:
        all_names.append(partition_name)
    donate = tuple(range(n_params, n_params + len(out_names)))

    def _body(*args):
        operands = list(args)
        if partition_name is not None:
            operands.append(bass2jax.partition_id_tensor())
        outs = bass2jax._bass_exec_p.bind(
            *operands,
            out_avals=tuple(out_avals),
            in_names=tuple(all_names),
            out_names=tuple(out_names),
            lowering_input_output_aliases=(),
            sim_require_finite=True,
            sim_require_nnan=True,
            nc=nc,
        )
        return tuple(outs)

    jitted = jax.jit(_body, donate_argnums=donate, keep_unused=True)
    zero_shapes = [(tuple(a.shape), a.dtype) for a in out_avals]

    def run(in_by_name):
        args = [in_by_name[nm] for nm in in_names]
        args += [np.zeros(s, d) for s, d in zero_shapes]
        outs = jitted(*args)
        return [np.asarray(o) for o in outs]

    return run


def get_runner():
    global _RUN
    if _RUN is None:
        _RUN = _build_runner()
    return _RUN


_NIB_LUT = None


def _nib_lut():
    """bf16-pattern -> 4-bit quant level k = clip(round(v/QSTEP + 7.5), 0, 15).
    One gather on the truncated-bf16 pattern quantizes 512K preds in ~2 ms."""
    global _NIB_LUT
    if _NIB_LUT is None:
        v = (np.arange(65536, dtype=np.uint32) << 16).view(np.float32)
        with np.errstate(invalid="ignore"):
            k = np.round(v.astype(np.float64) / QSTEP + 7.5)
        k = np.nan_to_num(k, nan=7.0, posinf=15.0, neginf=0.0)
        _NIB_LUT = np.clip(k, 0, 15).astype(np.uint8)
    return _NIB_LUT


def kernel(pred_sdt: np.ndarray, target_seg: np.ndarray) -> np.ndarray:
    run = get_runner()
    pred = np.ascontiguousarray(np.asarray(pred_sdt), dtype=np.float32)
    idx = (pred.view(np.uint32) >> 16).astype(np.uint16)
    nib = _nib_lut()[idx].reshape(B * H, W // 2, 2)
    pred8 = nib[:, :, 0] | (nib[:, :, 1] << 4)
    # target_seg channel 0 holds exact 0.0f/1.0f; byte 2 of the f32 LE
    # pattern (0x80 for 1.0) is a nonzero test packbits accepts directly.
    segb = np.asarray(target_seg).view(np.uint8)
    ch0_nz = segb.reshape(B, 3, H, W, 4)[:, 0, :, :, 2]
    bits = np.packbits(ch0_nz, bitorder="little").reshape(B * H, 32)
    (out,) = run({"pred": pred8, "ch0b": bits})
    return np.float32(float(out.sum()) / (B * H * W))
